# revision 19
# baseline (speedup 1.0000x reference)
"""Trainium2 Bass kernel for nn_Attention_60000693125929.

RMSNorm -> fused QKV proj -> interleaved RoPE -> causal attention -> out proj.
x: [4, 2048, 2048] f32.  8 NeuronCores: shard batch (4) x head-group (2x8 heads).

Per-core dataflow (bf16 matmul inputs, fp32 PSUM accumulation):
  1. Pass A streams x computing row sum-of-squares (ACT Square+accum);
     rinv = 1/sqrt(mean+eps).  Pass B re-streams x, scales+casts to bf16
     (DVE per-partition scalar), transposes via PE identity transposes into
     xsT quarters, and immediately computes V = xs @ Wv, spilled to DRAM.
  2. qT/kT = W^T-stationary matmuls ([e, n] layout, dh on partitions);
     interleaved RoPE via a constant rotation-permutation matmul + cos/sin
     elementwise.  q/k e-tiles emitted interleaved so attention can start.
  3. Per (i-chunk, head): S^T[j,i] blocks = kT.T @ qT; causal handled by
     skipping j>i blocks, adding a -1e30 triangle into PSUM via a constant
     matmul on diagonal blocks, and persistent zero-rect exp tiles.  exp on
     ACT (no max subtraction; |scores| ~ 5 so fp32 exp is safe), AV and
     ones-matmul rowsum accumulate in PSUM, normalized with fast reciprocal.
  4. out = outT.T @ WoutT per i-quarter, interleaved under phase 3; host
     sums the two head-group partials per batch.
"""
import numpy as np
import ml_dtypes
from contextlib import ExitStack

import concourse.bass as bass
import concourse.tile as tile
from concourse import bacc, mybir
from concourse.bass_utils import run_bass_kernel_spmd

F32 = mybir.dt.float32
BF16 = mybir.dt.bfloat16
AF = mybir.ActivationFunctionType
OP = mybir.AluOpType

B, N, D, H, DH = 4, 2048, 2048, 16, 128
HPC = 8                 # heads per core
EQK = 2 * HPC * DH      # 2048 q+k columns per core
EV = HPC * DH           # 1024 v columns per core
EPS = 1.1920929e-07
SCALE = DH ** -0.5
NT = N // 128           # 16 n-tiles
DT = D // 128           # 16 d-tiles
NCH = N // 512          # 4 n-chunks

_NC_CACHE = {}


def build_nc():
    if "nc" in _NC_CACHE:
        return _NC_CACHE["nc"]
    nc = bacc.Bacc("TRN2", target_bir_lowering=False, debug=False)

    x = nc.dram_tensor("x", [N, D], F32, kind="ExternalInput").ap()
    wqk = nc.dram_tensor("wqk", [D, EQK], BF16, kind="ExternalInput").ap()
    wv = nc.dram_tensor("wv", [D, EV], BF16, kind="ExternalInput").ap()
    wout = nc.dram_tensor("wout", [EV, D], BF16, kind="ExternalInput").ap()
    cos_d = nc.dram_tensor("cos_t", [DH, N], BF16, kind="ExternalInput").ap()
    sin_d = nc.dram_tensor("sin_t", [DH, N], BF16, kind="ExternalInput").ap()
    pm_d = nc.dram_tensor("pm", [DH, DH], BF16, kind="ExternalInput").ap()
    id_d = nc.dram_tensor("ident", [128, 128], BF16, kind="ExternalInput").ap()
    on_d = nc.dram_tensor("onesm", [128, 128], BF16, kind="ExternalInput").ap()
    mtri_d = nc.dram_tensor("mtri", [128, 128], BF16, kind="ExternalInput").ap()
    y = nc.dram_tensor("y", [N, D], F32, kind="ExternalOutput").ap()
    # DRAM scratch for V ([nt, 128, EV] bf16)
    Vd = nc.dram_tensor("Vspill", [NT, 128, EV], BF16).ap()

    with tile.TileContext(nc) as tc, ExitStack() as ctx:
        const_p = ctx.enter_context(tc.tile_pool(name="const", bufs=1))
        small_p = ctx.enter_context(tc.tile_pool(name="small", bufs=1))
        psum = ctx.enter_context(tc.tile_pool(name="psum", bufs=3, space="PSUM"))
        psav = ctx.enter_context(tc.tile_pool(name="psav", bufs=1, space="PSUM"))

        cos_s = const_p.tile([DH, N], BF16, tag="cos")
        sin_s = const_p.tile([DH, N], BF16, tag="sin")
        pm_s = const_p.tile([DH, DH], BF16, tag="pm")
        id_s = const_p.tile([128, 128], BF16, tag="ident")
        on_s = const_p.tile([128, 128], BF16, tag="ones")
        mtri_s = const_p.tile([128, 128], BF16, tag="mtri")
        cosr = const_p.tile([DH, N], BF16, tag="cosr")
        sinr = const_p.tile([DH, N], BF16, tag="sinr")
        nc.gpsimd.dma_start(id_s[:], id_d)
        nc.gpsimd.dma_start(cos_s[:], cos_d)
        nc.gpsimd.dma_start(sin_s[:], sin_d)
        nc.gpsimd.dma_start(pm_s[:], pm_d)
        nc.gpsimd.dma_start(on_s[:], on_d)
        nc.gpsimd.dma_start(mtri_s[:], mtri_d)

        ssq = small_p.tile([128, NT], F32, tag="ssq")
        rms = small_p.tile([128, NT], F32, tag="rms")
        rinv = small_p.tile([128, NT], F32, tag="rinv")
        rinvb = small_p.tile([128, NT], BF16, tag="rinvb")
        eps_s = small_p.tile([128, 1], F32, tag="eps")
        nc.vector.memzero(eps_s[:])
        nc.vector.tensor_scalar_add(eps_s[:], eps_s[:], EPS)

        # long-lived: qkT e-tiles (q: 0..7, k: 8..15)
        qk_p = ctx.enter_context(tc.tile_pool(name="qk", bufs=1))
        qkT = [qk_p.tile([128, N], BF16, tag=f"qkT{et}", name=f"qkT{et}")
               for et in range(16)]

        # ---- phase 1: RMSNorm + transpose + V --------------------------
        with tc.tile_pool(name="xsTp", bufs=1) as xsT_p:
            xsT = [xsT_p.tile([128, DT, 512], BF16, tag=f"xsT{q}",
                               name=f"xsT{q}")
                   for q in range(4)]
            with tc.tile_pool(name="ph1", bufs=2) as ph1_p, \
                 tc.tile_pool(name="ph1b", bufs=2) as ph1b_p, \
                 tc.tile_pool(name="wvp", bufs=1) as wv_p, \
                 tc.tile_pool(name="vbuf", bufs=2) as vb_p:
                wv_s = wv_p.tile([128, DT, EV], BF16, tag="wv")
                nc.sync.dma_start(wv_s[:],
                                  wv.rearrange("(dt p) e -> p dt e", p=128))
                # single pass: cast, transpose, Gram sumsq, V matmuls
                x_q = [nc.sync, nc.scalar]
                for t in range(NT):
                    q, tq = t // 4, t % 4
                    xt = ph1_p.tile([128, D], F32, tag="xin")
                    x_q[t % 2].dma_start(xt[:], x[t * 128:(t + 1) * 128, :])
                    xr = ph1b_p.tile([128, D], BF16, tag="xraw")
                    nc.vector.tensor_copy(xr[:], xt[:])
                    for q4 in range(4):
                        pt = psum.tile([128, 4, 128], BF16, tag="mm2")
                        for j in range(4):
                            dt_i = 4 * q4 + j
                            nc.tensor.transpose(
                                pt[:, j, :],
                                xr[:, dt_i * 128:(dt_i + 1) * 128], id_s[:])
                        nc.vector.tensor_copy(
                            xsT[q][:, 4 * q4:4 * q4 + 4,
                                   tq * 128:(tq + 1) * 128],
                            pt[:])
                    # Gram diag = row sum-of-squares (contraction over d)
                    pg = psav.tile([128, 128], F32, tag="rs")
                    for dt_i in range(DT):
                        blk = xsT[q][:, dt_i, tq * 128:(tq + 1) * 128]
                        nc.tensor.matmul(pg[:], blk, blk,
                                         start=(dt_i == 0),
                                         stop=(dt_i == DT - 1))
                    gsc = vb_p.tile([128, 128], BF16, tag="gsc")
                    nc.vector.scalar_tensor_tensor(
                        out=gsc[:], in0=pg[:], scalar=1.0, in1=id_s[:],
                        op0=OP.bypass, op1=OP.mult,
                        accum_out=ssq[:, t:t + 1])
                    nc.scalar.activation(rms[:, t:t + 1], ssq[:, t:t + 1],
                                         AF.Sqrt, bias=eps_s[:],
                                         scale=1.0 / D)
                    nc.vector.reciprocal(rinv[:, t:t + 1], rms[:, t:t + 1])
                    nc.vector.tensor_copy(rinvb[:, t:t + 1], rinv[:, t:t + 1])
                    for ech in range(EV // 512):
                        pv = psum.tile([128, 512], F32, tag="mm2")
                        for dt_i in range(DT):
                            nc.tensor.matmul(
                                pv[:],
                                xsT[q][:, dt_i, tq * 128:(tq + 1) * 128],
                                wv_s[:, dt_i, ech * 512:(ech + 1) * 512],
                                start=(dt_i == 0), stop=(dt_i == DT - 1))
                        vb = vb_p.tile([128, 512], BF16, tag="vb")
                        nc.vector.tensor_scalar_mul(vb[:], pv[:],
                                                    rinv[:, t:t + 1])
                        nc.gpsimd.dma_start(
                            Vd[t, :, ech * 512:(ech + 1) * 512], vb[:])
                # rinv -> row form -> cos/sin tables pre-scaled by rinv[n]
                for c in range(NCH):
                    prow = psav.tile([1, 512], F32, tag="av")
                    for tq in range(4):
                        t = 4 * c + tq
                        nc.tensor.matmul(
                            prow[:, tq * 128:(tq + 1) * 128],
                            rinvb[:, t:t + 1], id_s[:],
                            start=True, stop=True)
                    rrow = vb_p.tile([1, 512], BF16, tag="rrow")
                    nc.vector.tensor_copy(rrow[:], prow[:])
                    pb = psav.tile([128, 512], F32, tag="av")
                    nc.tensor.matmul(pb[:], on_s[0:1, :], rrow[:],
                                     start=True, stop=True)
                    sl = slice(c * 512, (c + 1) * 512)
                    nc.vector.tensor_tensor(cosr[:, sl], cos_s[:, sl],
                                            pb[:], OP.mult)
                    nc.vector.tensor_tensor(sinr[:, sl], sin_s[:, sl],
                                            pb[:], OP.mult)

            # ---- phase 2: qkT + RoPE (inside xsT scope) -----------------
            with tc.tile_pool(name="wqkp", bufs=2) as wqk_p, \
                 tc.tile_pool(name="rope", bufs=4) as rope_p:
                order = [v for pair in zip(range(8), range(8, 16))
                         for v in pair]
                for et in order:
                    wt = wqk_p.tile([128, DT, 128], BF16, tag="wqk")
                    nc.sync.dma_start(
                        wt[:],
                        wqk[:, et * 128:(et + 1) * 128]
                        .rearrange("(dt p) e -> p dt e", p=128))
                    for nch in range(NCH):
                        n0 = nch * 512
                        pq = psum.tile([128, 512], F32, tag="mm2")
                        for dt_i in range(DT):
                            nc.tensor.matmul(
                                pq[:], wt[:, dt_i, :],
                                xsT[nch][:, dt_i, :],
                                start=(dt_i == 0), stop=(dt_i == DT - 1))
                        raw = rope_p.tile([128, 512], BF16, tag="raw")
                        nc.scalar.activation(raw[:], pq[:], AF.Copy,
                                             bias=0.0, scale=1.0)
                        prot = psum.tile([128, 512], F32, tag="mm2")
                        nc.tensor.matmul(prot[:], pm_s[:], raw[:],
                                         start=True, stop=True)
                        t1 = rope_p.tile([128, 512], BF16, tag="t1")
                        nc.vector.tensor_tensor(
                            t1[:], raw[:], cosr[:, n0:n0 + 512], OP.mult)
                        t2 = rope_p.tile([128, 512], BF16, tag="t2")
                        nc.vector.tensor_tensor(
                            t2[:], prot[:], sinr[:, n0:n0 + 512], OP.mult)
                        nc.vector.tensor_add(
                            qkT[et][:, n0:n0 + 512], t1[:], t2[:])

        # ---- phase 3+4: causal attention + out projection ---------------
        with tc.tile_pool(name="outp", bufs=1) as out_p, \
             tc.tile_pool(name="exps", bufs=6) as exps_p, \
             tc.tile_pool(name="espp", bufs=1) as esp_p, \
             tc.tile_pool(name="att", bufs=2) as att_p, \
             tc.tile_pool(name="vstr", bufs=2) as vs_p, \
             tc.tile_pool(name="woutp", bufs=1) as wo_p, \
             tc.tile_pool(name="ybufp", bufs=2) as y_p:
            wo_s = wo_p.tile([128, HPC, D], BF16, tag="wo")
            nc.sync.dma_start(
                wo_s[:], wout.rearrange("(et p) d -> p et d", p=128))
            outT = [out_p.tile([128, HPC, 512], BF16, tag=f"outT{q}",
                                name=f"outT{q}")
                    for q in range(4)]
            esp = []
            for r in range(4):
                e_ = esp_p.tile([128, 512], BF16, tag=f"esp{r}")
                if r > 0:
                    nc.vector.memzero(e_[:, :r * 128])
                esp.append(e_)
            for ic in range(NCH):
                i0 = ic * 512
                njt = 4 * ic + 4
                for h in range(HPC):
                    vstrip = vs_p.tile([128, NT, 128], BF16, tag="vstr")
                    nc.sync.dma_start(
                        vstrip[:, :njt, :],
                        Vd[:njt, :, h * 128:(h + 1) * 128]
                        .rearrange("jt p e -> p jt e"))
                    po = psav.tile([128, 512], F32, tag="av")
                    pr = psav.tile([128, 512], F32, tag="rs")
                    # full tiles in pairs: jt in [0, 4*ic)
                    for jp in range(0, 4 * ic, 2):
                        psq = psum.tile([128, 1024], F32, tag="mm2")
                        es = exps_p.tile([128, 1024], BF16, tag="es")
                        for half in range(2):
                            jt = jp + half
                            nc.tensor.matmul(
                                psq[:, half * 512:(half + 1) * 512],
                                qkT[HPC + h][:, jt * 128:(jt + 1) * 128],
                                qkT[h][:, i0:i0 + 512],
                                start=True, stop=True)
                        nc.scalar.activation(es[:], psq[:], AF.Exp,
                                             bias=0.0, scale=SCALE)
                        for half in range(2):
                            jt = jp + half
                            sl = slice(half * 512, (half + 1) * 512)
                            nc.tensor.matmul(
                                po[:], vstrip[:, jt, :], es[:, sl],
                                start=(jt == 0), stop=False)
                            nc.tensor.matmul(
                                pr[:], on_s[:], es[:, sl],
                                start=(jt == 0), stop=False)
                    # diagonal partial tiles: jt = 4*ic + r
                    for r in range(4):
                        jt = 4 * ic + r
                        lo = 128 * r
                        psq = psum.tile([128, 512], F32, tag="mm2")
                        nc.tensor.matmul(
                            psq[:, :512],
                            qkT[HPC + h][:, jt * 128:(jt + 1) * 128],
                            qkT[h][:, i0:i0 + 512],
                            start=True, stop=False)
                        nc.tensor.matmul(
                            psq[:, lo:lo + 128], mtri_s[:], id_s[:],
                            start=False, stop=True)
                        es = esp[r]
                        nc.scalar.activation(es[:, lo:], psq[:, lo:512],
                                             AF.Exp, bias=0.0, scale=SCALE)
                        nc.tensor.matmul(
                            po[:], vstrip[:, jt, :], es[:],
                            start=(jt == 0), stop=(jt == njt - 1))
                        nc.tensor.matmul(
                            pr[:], on_s[:], es[:],
                            start=(jt == 0), stop=(jt == njt - 1))
                    rec = att_p.tile([128, 512], F32, tag="rec")
                    rsc = att_p.tile([128, 512], F32, tag="rsc")
                    nc.vector.reciprocal_approx_accurate(rec[:], pr[:], rsc[:])
                    nc.vector.tensor_tensor(
                        outT[ic][:, h, :], po[:], rec[:], OP.mult)
                # out projection for this i-quarter
                for tq in range(4):
                    t = 4 * ic + tq
                    yb = y_p.tile([128, D], F32, tag="yb")
                    for dch in range(4):
                        py = psum.tile([128, 512], F32, tag="mm2")
                        for et in range(HPC):
                            nc.tensor.matmul(
                                py[:],
                                outT[ic][:, et, tq * 128:(tq + 1) * 128],
                                wo_s[:, et, dch * 512:(dch + 1) * 512],
                                start=(et == 0), stop=(et == HPC - 1))
                        nc.vector.tensor_copy(
                            yb[:, dch * 512:(dch + 1) * 512], py[:, :512])
                    nc.sync.dma_start(y[t * 128:(t + 1) * 128, :], yb[:])

    nc.compile()
    _NC_CACHE["nc"] = nc
    return nc


def _host_prep(rotary_pos_emb, w_rms, w_qkv, w_out):
    bf = ml_dtypes.bfloat16
    cos_t = np.ascontiguousarray(np.cos(rotary_pos_emb).T).astype(bf)
    sin_t = np.ascontiguousarray(np.sin(rotary_pos_emb).T).astype(bf)
    # rotate_half as a matrix: rot(t)[2i] = -t[2i+1], rot(t)[2i+1] = t[2i]
    P = np.zeros((DH, DH), np.float32)
    for i in range(DH // 2):
        P[2 * i, 2 * i + 1] = -1.0
        P[2 * i + 1, 2 * i] = 1.0
    pm = np.ascontiguousarray(P.T).astype(bf)       # lhsT for rot matmul
    ident = np.eye(128, dtype=bf)
    onesm = np.ones((128, 128), dtype=bf)
    # mtri = M.T with M[jj, cc] = -1e30 where cc < jj (strict lower tri)
    M = np.where(np.arange(128)[None, :] < np.arange(128)[:, None],
                 np.float32(-1e30), np.float32(0.0))
    mtri = np.ascontiguousarray(M.T).astype(bf)

    Ws = (w_qkv * w_rms[None, :]).astype(np.float32)  # fold RMSNorm weight
    per_core = []
    for g in range(2):
        rq = Ws[g * 1024:(g + 1) * 1024]              # q rows, heads 8g..
        rk = Ws[D + g * 1024:D + (g + 1) * 1024]      # k rows
        rv = Ws[2 * D + g * 1024:2 * D + (g + 1) * 1024]
        wqk_g = np.ascontiguousarray(np.concatenate([rq, rk], 0).T).astype(bf)
        wv_g = np.ascontiguousarray(rv.T).astype(bf)
        wout_g = np.ascontiguousarray(w_out[:, g * 1024:(g + 1) * 1024].T
                                      ).astype(bf)
        per_core.append(dict(wqk=wqk_g, wv=wv_g, wout=wout_g, cos_t=cos_t,
                             sin_t=sin_t, pm=pm, ident=ident, onesm=onesm,
                             mtri=mtri))
    return per_core


def kernel(x, rotary_pos_emb, w_rms, w_qkv, w_out, _run=None):
    x = np.asarray(x, np.float32)
    rotary_pos_emb = np.asarray(rotary_pos_emb, np.float32)
    w_rms = np.asarray(w_rms, np.float32)
    w_qkv = np.asarray(w_qkv, np.float32)
    w_out = np.asarray(w_out, np.float32)

    nc = build_nc()
    groups = _host_prep(rotary_pos_emb, w_rms, w_qkv, w_out)
    in_maps = []
    for b in range(B):
        for g in range(2):
            m = dict(groups[g])
            m["x"] = np.ascontiguousarray(x[b])
            in_maps.append(m)
    if _run is None:
        res = run_bass_kernel_spmd(nc, in_maps, core_ids=list(range(8)))
        results = res.results
    else:
        results = _run(nc, in_maps)

    y = np.empty((B, N, D), np.float32)
    for b in range(B):
        y[b] = results[2 * b]["y"] + results[2 * b + 1]["y"]
    return y
